# revision 6
# baseline (speedup 1.0000x reference)
"""GaussianNB log-posterior kernel for 8 Trainium2 NeuronCores.

out[b, c] = log_pi[c] - 0.5 * sum_f(log2pi + log_var[c,f] + (x[b,f]-mu[c,f])^2 / var[c,f])
          = const_c + sum_f (-0.5*x^2)[b,f]*inv[c,f] + sum_f x[b,f]*wc[c,f]
  with inv = exp(-lv), wc = mu*inv,
       const_c = -0.5*(sum_f lv + sum_f mu*wc - 2*lp_c) - 0.5*F*log2pi

Strategy: data-parallel over batch (B=2048 -> 256 rows/core), weights replicated.
Wire format fp16, f-major (host does layout only: cast + transpose + pack; all
arithmetic on device). Per core, 6 pipelined 0.25MB DMA chunks ordered
lv0,x0,mu0,x1,lv1,mu1 to match the dependency chains. PE runs cheap warmup
matmuls during the DMA window (p-state ramp needs ~3us of continuous busy),
then 32 fp16 GEMM matmuls (stationary = x/-0.5x^2 b-chunks, moving = inv/wc
tiles). const_c is folded into the same PSUM accumulation: sum_f reductions
via a shared ones-column stationary, lp via a -2-scalar K=1 matmul, and a
final ones-row K=1 matmul broadcasts const into the output tiles.
Output (256b, 256c) fp16, host casts to f32.
"""
import sys

sys.path.insert(0, "/opt/trn_rl_repo")
import numpy as np
import concourse.bacc as bacc
import concourse.mybir as mybir
from concourse.tile import TileContext
from concourse.bass_utils import run_bass_kernel_spmd

B, C, F = 2048, 256, 1024
NCORES = 8
BSH = B // NCORES  # 256
KT = F // 128      # 8 k-tiles
LOG_2PI = float(np.log(2.0 * np.pi))
F32 = mybir.dt.float32
F16 = mybir.dt.float16
OP = mybir.AluOpType
AF = mybir.ActivationFunctionType
NWARM = 40

_CACHE = {}


def _build():
    nc = bacc.Bacc("TRN2", target_bir_lowering=False, debug=False, num_devices=NCORES)
    # f-major packed inputs: [:, k*256:(k+1)*256] = rows k*128..(k+1)*128 of the
    # (F, ...) transposed tensor. mu carries lp as a (1, 256) row on partition 0.
    lv_d = nc.dram_tensor("lvt", [128, 2 * F], F16, kind="ExternalInput").ap()
    x_d = nc.dram_tensor("xt", [128, 2 * F], F16, kind="ExternalInput").ap()
    mu_d = nc.dram_tensor("mut", [128, 2 * F + 256], F16, kind="ExternalInput").ap()
    out_d = nc.dram_tensor("out", [128, 2 * BSH], F16, kind="ExternalOutput").ap()

    with TileContext(nc) as tc:
        with (
            tc.tile_pool(name="sb", bufs=1) as sb,
            tc.tile_pool(name="po", bufs=1, space="PSUM") as po,
        ):
            lvt = sb.tile([128, 2 * F], F16, tag="lvt")
            xt = sb.tile([128, 2 * F], F16, tag="xt")
            mut = sb.tile([128, 2 * F + 256], F16, tag="mut")
            h0, h1 = slice(0, F), slice(F, 2 * F)
            # chunk order matches consumer chains: exp(lv0) is the longest
            # pole, then x0 (square), mu0 (wc/m2i), x1, lv1, mu1(+lp)
            nc.sync.dma_start(out=lvt[:, h0], in_=lv_d[:, h0])
            nc.sync.dma_start(out=xt[:, h0], in_=x_d[:, h0])
            nc.sync.dma_start(out=mut[:, h0], in_=mu_d[:, h0])
            nc.sync.dma_start(out=xt[:, h1], in_=x_d[:, h1])
            nc.sync.dma_start(out=lvt[:, h1], in_=lv_d[:, h1])
            nc.sync.dma_start(out=mut[:, F:2 * F + 256], in_=mu_d[:, F:2 * F + 256])
            lp_row = mut[0:1, 2 * F:2 * F + 256]

            # constants + PE warmup during the DMA window
            ones_col = sb.tile([128, 1], F16, tag="onc")
            ones_row = sb.tile([1, 128], F16, tag="onr")
            neg2 = sb.tile([1, 1], F16, tag="n2")
            dmy = sb.tile([1, 128], F16, tag="dmy")
            nc.gpsimd.memset(ones_col[:], 1.0)
            nc.gpsimd.memset(ones_row[:], 1.0)
            nc.gpsimd.memset(neg2[:], -2.0)
            nc.gpsimd.memset(dmy[:], 0.5)
            tw = sb.tile([1, 1], F32, tag="tw")
            tw2 = sb.tile([1, 1], F32, tag="tw2")
            nc.gpsimd.memset(tw[:], 0.0)
            nc.scalar.activation(tw2[:], tw[:], AF.Exp)  # preload exp table
            wp = po.tile([128, 128], F32, tag="wp")
            for i in range(NWARM):
                nc.tensor.matmul(wp[:], dmy[:], dmy[:], start=True, stop=True)

            # ---- prep: inv = exp(-lv); x2 = (-0.5x)*x; wc = mu*inv; m2i = mu*wc
            invt = sb.tile([128, 2 * F], F16, tag="invt")
            nhxt = sb.tile([128, 2 * F], F16, tag="nhxt")
            x2t = sb.tile([128, 2 * F], F16, tag="x2t")
            wct = sb.tile([128, 2 * F], F16, tag="wct")
            m2it = sb.tile([128, 2 * F], F16, tag="m2it")
            for h in (h0, h1):
                nc.scalar.activation(invt[:, h], lvt[:, h], AF.Exp, scale=-1.0)
                nc.vector.tensor_scalar_mul(nhxt[:, h], xt[:, h], -0.5)
                nc.vector.tensor_mul(x2t[:, h], nhxt[:, h], xt[:, h])
                nc.vector.tensor_mul(wct[:, h], mut[:, h], invt[:, h])
                nc.vector.tensor_mul(m2it[:, h], mut[:, h], wct[:, h])

            # ---- GEMMs + folded const into 2 b-half PSUM tiles ----
            x3 = xt[:].rearrange("p (k n) -> p k n", k=KT)
            x23 = x2t[:].rearrange("p (k n) -> p k n", k=KT)
            iv3 = invt[:].rearrange("p (k n) -> p k n", k=KT)
            wc3 = wct[:].rearrange("p (k n) -> p k n", k=KT)
            lv3 = lvt[:].rearrange("p (k n) -> p k n", k=KT)
            m23 = m2it[:].rearrange("p (k n) -> p k n", k=KT)
            pg = [po.tile([128, C], F32, tag=f"pg{bh}", name=f"pg{bh}") for bh in range(2)]
            s_ps = po.tile([1, C], F32, tag="sps")
            step = [0, 0]
            rstep = 0

            def gemms(A3, W3, ks):
                for k in ks:
                    for bh in range(2):
                        nc.tensor.matmul(
                            pg[bh][:], A3[:, k, bh * 128:(bh + 1) * 128], W3[:, k, :],
                            start=(step[bh] == 0), stop=False, skip_group_check=True,
                        )
                        step[bh] += 1

            def reds(T3, ks):
                nonlocal rstep
                for k in ks:
                    nc.tensor.matmul(
                        s_ps[:], ones_col[:], T3[:, k, :],
                        start=(rstep == 0), stop=False, skip_group_check=True,
                    )
                    rstep += 1

            gemms(x23, iv3, range(0, 4))       # quad h0
            reds(lv3, range(0, 4))
            reds(m23, range(0, 4))
            gemms(x3, wc3, range(0, 4))        # cross h0
            gemms(x23, iv3, range(4, 8))       # quad h1
            reds(lv3, range(4, 8))
            gemms(x3, wc3, range(4, 8))        # cross h1
            reds(m23, range(4, 8))
            nc.tensor.matmul(s_ps[:], neg2[:], lp_row, start=False, stop=True,
                             skip_group_check=True)
            const_row = sb.tile([1, C], F16, tag="crow")
            nc.vector.tensor_scalar(const_row[:], s_ps[:], -0.5, -0.5 * F * LOG_2PI,
                                    OP.mult, OP.add)
            for bh in range(2):
                nc.tensor.matmul(pg[bh][:], ones_row[:], const_row[:],
                                 start=False, stop=True, skip_group_check=True)

            # ---- copy out + DMA ----
            out_sb = sb.tile([128, 2 * BSH], F16, tag="osb")
            nc.vector.tensor_copy(out_sb[:, 0:BSH], pg[0][:])
            nc.scalar.copy(out=out_sb[:, BSH:2 * BSH], in_=pg[1][:])
            nc.sync.dma_start(out=out_d[:, :], in_=out_sb[:])

    nc.compile()
    return nc


def get_nc():
    if "nc" not in _CACHE:
        _CACHE["nc"] = _build()
    return _CACHE["nc"]


def _pack_fmajor(aT):
    # (F=1024, n) f-major -> SBUF-packed (128, 8*n): cols k*n..(k+1)*n = rows
    # k*128..(k+1)*128
    Fdim, n = aT.shape
    k = Fdim // 128
    return np.ascontiguousarray(
        aT.reshape(k, 128, n).transpose(1, 0, 2).reshape(128, k * n)
    )


def make_in_maps(x, mu, log_var, log_pi):
    x16 = np.asarray(x, dtype=np.float16)
    mu16 = np.asarray(mu, dtype=np.float16)
    lv16 = np.asarray(log_var, dtype=np.float16)
    lp16 = np.asarray(log_pi, dtype=np.float16).reshape(1, C)

    lvt = _pack_fmajor(lv16.T)                      # (128, 2048)
    mut = _pack_fmajor(mu16.T)                      # (128, 2048)
    mut = np.concatenate([mut, np.zeros((128, 256), np.float16)], axis=1)
    mut[0:1, 2 * F:2 * F + 256] = lp16
    mut = np.ascontiguousarray(mut)
    xT = x16.T                                      # (1024, 2048)
    return [
        {"lvt": lvt, "mut": mut,
         "xt": _pack_fmajor(xT[:, c * BSH:(c + 1) * BSH])}
        for c in range(NCORES)
    ]


def unpack_out(res):
    out = np.empty((B, C), dtype=np.float32)
    for c in range(NCORES):
        o = res.results[c]["out"]                   # (128, 512) fp16
        out[c * BSH:c * BSH + 128, :] = o[:, 0:BSH]
        out[c * BSH + 128:(c + 1) * BSH, :] = o[:, BSH:2 * BSH]
    return out


def kernel(x, mu, log_var, log_pi):
    nc = get_nc()
    in_maps = make_in_maps(x, mu, log_var, log_pi)
    res = run_bass_kernel_spmd(nc, in_maps, list(range(NCORES)))
    return unpack_out(res)


# revision 13
# speedup vs baseline: 1.0049x; 1.0049x over previous
"""GaussianNB log-posterior kernel for 8 Trainium2 NeuronCores.

out[b, c] = log_pi[c] - 0.5 * sum_f(log2pi + log_var[c,f] + (x[b,f]-mu[c,f])^2 / var[c,f])
          = const_c + sum_f (-0.5*x^2)[b,f]*inv[c,f] + sum_f x[b,f]*wc[c,f]
  with inv = exp(-lv), wc = mu*inv,
       const_c = -0.5*(sum_f lv + sum_f mu*wc - 2*lp_c) - 0.5*F*log2pi

Strategy: data-parallel over batch (B=2048 -> 256 rows/core), weights replicated.
Wire format fp16, f-major (host does layout only: cast + transpose + pack; all
arithmetic on device). Per core, 6 pipelined 0.25MB DMA chunks ordered
lv0,x0,mu0,x1,lv1,mu1 to match the dependency chains. PE runs cheap warmup
matmuls during the DMA window (p-state ramp needs ~3us of continuous busy),
then 32 fp16 GEMM matmuls (stationary = x/-0.5x^2 b-chunks, moving = inv/wc
tiles). const_c is folded into the same PSUM accumulation: sum_f reductions
via a shared ones-column stationary, lp via a -2-scalar K=1 matmul, and a
final ones-row K=1 matmul broadcasts const into the output tiles.
Output (256b, 256c) fp16, host casts to f32.
"""
import sys

sys.path.insert(0, "/opt/trn_rl_repo")
import numpy as np
import concourse.bacc as bacc
import concourse.mybir as mybir
from concourse.tile import TileContext
from concourse.bass_utils import run_bass_kernel_spmd

B, C, F = 2048, 256, 1024
NCORES = 8
BSH = B // NCORES  # 256
KT = F // 128      # 8 k-tiles
LOG_2PI = float(np.log(2.0 * np.pi))
F32 = mybir.dt.float32
F16 = mybir.dt.float16
OP = mybir.AluOpType
AF = mybir.ActivationFunctionType
NWARM = 48

_CACHE = {}


def _build():
    nc = bacc.Bacc("TRN2", target_bir_lowering=False, debug=False, num_devices=NCORES)
    # f-major packed inputs: [:, k*256:(k+1)*256] = rows k*128..(k+1)*128 of the
    # (F, ...) transposed tensor. mu carries lp as a (1, 256) row on partition 0.
    lv_d = nc.dram_tensor("lvt", [128, 2 * F], F16, kind="ExternalInput").ap()
    x_d = nc.dram_tensor("xt", [128, 2 * F], F16, kind="ExternalInput").ap()
    mu_d = nc.dram_tensor("mut", [128, 2 * F + 256], F16, kind="ExternalInput").ap()
    out_d = nc.dram_tensor("out", [128, 2 * BSH], F16, kind="ExternalOutput").ap()

    with TileContext(nc) as tc:
        with (
            tc.tile_pool(name="sb", bufs=1) as sb,
            tc.tile_pool(name="po", bufs=1, space="PSUM") as po,
        ):
            lvt = sb.tile([128, 2 * F], F16, tag="lvt")
            xt = sb.tile([128, 2 * F], F16, tag="xt")
            mut = sb.tile([128, 2 * F + 256], F16, tag="mut")
            h0, h1 = slice(0, F), slice(F, 2 * F)
            # chunk order matches consumer chains: exp(lv0) is the longest
            # pole, then x0 (square), mu0 (wc/m2i), x1, lv1, mu1(+lp)
            nc.sync.dma_start(out=lvt[:, h0], in_=lv_d[:, h0])
            nc.sync.dma_start(out=xt[:, h0], in_=x_d[:, h0])
            nc.sync.dma_start(out=mut[:, h0], in_=mu_d[:, h0])
            nc.sync.dma_start(out=lvt[:, h1], in_=lv_d[:, h1])
            nc.sync.dma_start(out=xt[:, h1], in_=x_d[:, h1])
            nc.sync.dma_start(out=mut[:, F:2 * F + 256], in_=mu_d[:, F:2 * F + 256])
            lp_row = mut[0:1, 2 * F:2 * F + 256]

            # constants + PE warmup during the DMA window
            ones_col = sb.tile([128, 1], F16, tag="onc")
            neg2 = sb.tile([1, 1], F16, tag="n2")
            ones1 = sb.tile([1, 1], F16, tag="o1")
            dmy = sb.tile([1, 128], F16, tag="dmy")
            nh_row = sb.tile([1, 128], F16, tag="nhr")   # crow stationary: -0.5
            logc_row = sb.tile([1, C], F16, tag="logc")  # F*log2pi, folded into s
            s_row = sb.tile([1, C], F16, tag="srow")
            nc.gpsimd.memset(ones_col[:], 1.0)
            nc.gpsimd.memset(neg2[:], -2.0)
            nc.gpsimd.memset(ones1[:], 1.0)
            nc.gpsimd.memset(dmy[:], 0.5)
            nc.gpsimd.memset(nh_row[:], -0.5)
            nc.gpsimd.memset(logc_row[:], F * LOG_2PI)
            tw = sb.tile([1, 1], F32, tag="tw")
            tw2 = sb.tile([1, 1], F32, tag="tw2")
            nc.gpsimd.memset(tw[:], 0.0)
            nc.scalar.activation(tw2[:], tw[:], AF.Exp)  # preload exp table
            wp = po.tile([128, 128], F32, tag="wp")
            for i in range(NWARM):
                nc.tensor.matmul(wp[:], dmy[:], dmy[:], start=True, stop=True)

            def fillers(n):
                for _ in range(n):
                    nc.tensor.matmul(wp[:], dmy[:], dmy[:], start=True, stop=True)

            # ---- prep: inv = exp(-lv); x2 = (-0.5x)*x; wc = mu*inv; m2i = mu*wc
            invt = sb.tile([128, 2 * F], F16, tag="invt")
            nhxt = sb.tile([128, 2 * F], F16, tag="nhxt")
            x2t = sb.tile([128, 2 * F], F16, tag="x2t")
            wct = sb.tile([128, 2 * F], F16, tag="wct")
            m2it = sb.tile([128, 2 * F], F16, tag="m2it")
            for h in (h0, h1):
                nc.scalar.activation(invt[:, h], lvt[:, h], AF.Exp, scale=-1.0)
                nc.vector.tensor_scalar_mul(nhxt[:, h], xt[:, h], -0.5)
                nc.vector.tensor_mul(x2t[:, h], nhxt[:, h], xt[:, h])
                nc.vector.tensor_mul(wct[:, h], mut[:, h], invt[:, h])
                nc.vector.tensor_mul(m2it[:, h], mut[:, h], wct[:, h])

            # ---- GEMMs + folded const into 2 b-half PSUM tiles ----
            x3 = xt[:].rearrange("p (k n) -> p k n", k=KT)
            x23 = x2t[:].rearrange("p (k n) -> p k n", k=KT)
            iv3 = invt[:].rearrange("p (k n) -> p k n", k=KT)
            wc3 = wct[:].rearrange("p (k n) -> p k n", k=KT)
            lv3 = lvt[:].rearrange("p (k n) -> p k n", k=KT)
            m23 = m2it[:].rearrange("p (k n) -> p k n", k=KT)
            pg = [po.tile([128, C], F32, tag=f"pg{bh}", name=f"pg{bh}") for bh in range(2)]
            s_ps = po.tile([1, C], F32, tag="sps")
            step = [0, 0]
            rstep = 0

            def gemms(A3, W3, ks):
                for k in ks:
                    for bh in range(2):
                        nc.tensor.matmul(
                            pg[bh][:], A3[:, k, bh * 128:(bh + 1) * 128], W3[:, k, :],
                            start=(step[bh] == 0), stop=False, skip_group_check=True,
                        )
                        step[bh] += 1

            def reds(T3, ks):
                nonlocal rstep
                for k in ks:
                    nc.tensor.matmul(
                        s_ps[:], ones_col[:], T3[:, k, :],
                        start=(rstep == 0), stop=False, skip_group_check=True,
                    )
                    rstep += 1

            gemms(x23, iv3, range(0, 4))       # quad h0
            reds(lv3, range(0, 4))
            fillers(5)                         # bridge wait for wc h0
            gemms(x3, wc3, range(0, 4))        # cross h0
            reds(m23, range(0, 4))
            reds(lv3, range(4, 8))
            fillers(5)                         # bridge wait for x2 h1
            gemms(x23, iv3, range(4, 8))       # quad h1
            fillers(5)                         # bridge wait for wc h1
            gemms(x3, wc3, range(4, 8))        # cross h1
            reds(m23, range(4, 8))
            nc.tensor.matmul(s_ps[:], neg2[:], lp_row, start=False, stop=False,
                             skip_group_check=True)
            nc.tensor.matmul(s_ps[:], ones1[:], logc_row[:], start=False, stop=True,
                             skip_group_check=True)
            nc.scalar.copy(out=s_row[:], in_=s_ps[:])
            for bh in range(2):
                nc.tensor.matmul(pg[bh][:], nh_row[:], s_row[:],
                                 start=False, stop=True, skip_group_check=True)

            # ---- copy out + DMA ----
            out_sb = sb.tile([128, 2 * BSH], F16, tag="osb")
            nc.vector.tensor_copy(out_sb[:, 0:BSH], pg[0][:])
            nc.scalar.copy(out=out_sb[:, BSH:2 * BSH], in_=pg[1][:])
            nc.sync.dma_start(out=out_d[:, :], in_=out_sb[:])

    nc.compile()
    return nc


def get_nc():
    if "nc" not in _CACHE:
        _CACHE["nc"] = _build()
    return _CACHE["nc"]


def _pack_fmajor(aT):
    # (F=1024, n) f-major -> SBUF-packed (128, 8*n): cols k*n..(k+1)*n = rows
    # k*128..(k+1)*128
    Fdim, n = aT.shape
    k = Fdim // 128
    return np.ascontiguousarray(
        aT.reshape(k, 128, n).transpose(1, 0, 2).reshape(128, k * n)
    )


def make_in_maps(x, mu, log_var, log_pi):
    x16 = np.asarray(x, dtype=np.float16)
    mu16 = np.asarray(mu, dtype=np.float16)
    lv16 = np.asarray(log_var, dtype=np.float16)
    lp16 = np.asarray(log_pi, dtype=np.float16).reshape(1, C)

    lvt = _pack_fmajor(lv16.T)                      # (128, 2048)
    mut = _pack_fmajor(mu16.T)                      # (128, 2048)
    mut = np.concatenate([mut, np.zeros((128, 256), np.float16)], axis=1)
    mut[0:1, 2 * F:2 * F + 256] = lp16
    mut = np.ascontiguousarray(mut)
    xT = x16.T                                      # (1024, 2048)
    return [
        {"lvt": lvt, "mut": mut,
         "xt": _pack_fmajor(xT[:, c * BSH:(c + 1) * BSH])}
        for c in range(NCORES)
    ]


def unpack_out(res):
    out = np.empty((B, C), dtype=np.float32)
    for c in range(NCORES):
        o = res.results[c]["out"]                   # (128, 512) fp16
        out[c * BSH:c * BSH + 128, :] = o[:, 0:BSH]
        out[c * BSH + 128:(c + 1) * BSH, :] = o[:, BSH:2 * BSH]
    return out


def kernel(x, mu, log_var, log_pi):
    nc = get_nc()
    in_maps = make_in_maps(x, mu, log_var, log_pi)
    res = run_bass_kernel_spmd(nc, in_maps, list(range(NCORES)))
    return unpack_out(res)
